# revision 17
# baseline (speedup 1.0000x reference)
"""Local-window attention encoder layer on 8 Trainium2 cores.

Problem: B=4, S=8192, D=512, window W=128, H=8 heads (HD=64), FF dim 2048.
Sharding: [B*nW]=256 independent windows split 32/core across 8 cores.

Per-core device kernel, restructured for engine balance (v2):
  4 windows (512 tokens) per iteration so every big matmul streams a
  512-wide moving operand.  Scores are produced TRANSPOSED ([k,q]) so
  softmax exp needs no per-row accumulation: exp runs as one batched
  ACT op per head over all 4 windows, and the softmax denominator rides
  the attention matmul itself via a ones-column appended to V (the
  probs stationary contracts over k, so an extra moving column of ones
  yields sum_k p[k,q] in the same instruction).  Normalization is a
  per-partition divide during the PSUM->SBUF read of the token-major
  attention output.  LayerNorm rstd uses exp(-0.5*ln(var+eps)) so the
  ACT engine stays on the natural_log_exp table set for the whole
  kernel (no table reloads); FF1's relu+bias also runs on ACT (relu is
  in every set).  LN gains/biases are folded host-side where linear
  algebra allows (g1 into W1, b1+=W1@ln1_b, b2+=ln1_b, out_b+=Wo@bv).
"""

import numpy as np
import ml_dtypes

import concourse.bass as bass
import concourse.tile as tile
from concourse import bacc, mybir
from concourse.bass_utils import run_bass_kernel_spmd

BF16 = ml_dtypes.bfloat16
F32 = mybir.dt.float32
BF = mybir.dt.bfloat16
AF = mybir.ActivationFunctionType
ALU = mybir.AluOpType

D = 512
H = 8
W = 128
HD = 64
FF = 2048
EPS = 1e-5
N_CORES = 8
B, S = 4, 8192
NW_TOT = (B * S) // W          # 256 windows
WPC = NW_TOT // N_CORES        # 32 windows per core
G = 4                          # windows per iteration
ITERS = WPC // G               # 8 iterations
KC = D // 128                  # 4 contraction chunks of 128
FC = FF // 128                 # 16 ff chunks
VP = 68                        # v row pitch (64 chans + ones col + pad)


def _build_nc(n_iters=ITERS):
    nc = bacc.Bacc("TRN2", target_bir_lowering=False, debug=False,
                   num_devices=N_CORES)
    n_tok = n_iters * G * W

    x_d = nc.dram_tensor("x", [n_tok, D], F32, kind="ExternalInput").ap()
    out_d = nc.dram_tensor("out", [n_tok, D], F32, kind="ExternalOutput").ap()
    wqk_d = nc.dram_tensor("wqk", [128, KC * 1024], BF, kind="ExternalInput").ap()
    wv_d = nc.dram_tensor("wv", [128, KC * D], BF, kind="ExternalInput").ap()
    wo_d = nc.dram_tensor("wo", [128, KC * D], BF, kind="ExternalInput").ap()
    w1_d = nc.dram_tensor("w1t", [128, KC * FF], BF, kind="ExternalInput").ap()
    w2_d = nc.dram_tensor("w2t", [128, FC * D], BF, kind="ExternalInput").ap()
    qkb_d = nc.dram_tensor("qkb", [128, 8], F32, kind="ExternalInput").ap()
    b1_d = nc.dram_tensor("b1t", [128, FC], F32, kind="ExternalInput").ap()
    ob_d = nc.dram_tensor("obc", [128, D], F32, kind="ExternalInput").ap()
    b2_d = nc.dram_tensor("b2r", [1, D], BF, kind="ExternalInput").ap()
    g1_d = nc.dram_tensor("g1b", [128, D], F32, kind="ExternalInput").ap()
    g2_d = nc.dram_tensor("g2b", [128, D], F32, kind="ExternalInput").ap()
    bb2_d = nc.dram_tensor("bb2", [128, D], F32, kind="ExternalInput").ap()
    on_d = nc.dram_tensor("ones1", [1, 128], BF, kind="ExternalInput").ap()
    idb_d = nc.dram_tensor("idb", [128, 128], BF, kind="ExternalInput").ap()
    idf_d = nc.dram_tensor("idf", [128, 128], F32, kind="ExternalInput").ap()

    xv = x_d.rearrange("(w p) d -> w p d", p=W)
    ov = out_d.rearrange("(w p) d -> w p d", p=W)

    with tile.TileContext(nc) as tc:
        with (
            tc.tile_pool(name="const", bufs=1) as cp,
            tc.tile_pool(name="stream", bufs=2) as sp,
            tc.tile_pool(name="pr3", bufs=3) as sp3,
            tc.tile_pool(name="big", bufs=3, space="PSUM") as pbig,
            tc.tile_pool(name="sc", bufs=2, space="PSUM") as psc,
            tc.tile_pool(name="pat", bufs=1, space="PSUM") as ppat,
            tc.tile_pool(name="tp", bufs=2, space="PSUM") as ptp,
        ):
            # ---- resident constants ----
            # first iteration's activations + small consts first so the PE
            # can start transposing while the big weights stream in
            idf = cp.tile([128, 128], F32); nc.sync.dma_start(idf[:], idf_d[:])
            x0 = sp.tile([128, G, D], F32, tag="x", name="x0")
            for w0_ in range(G):
                nc.sync.dma_start(x0[:, w0_, :], xv[w0_])
            wqk = cp.tile([128, KC, 1024], BF); nc.sync.dma_start(wqk[:], wqk_d[:])
            wv = cp.tile([128, KC, D], BF); nc.sync.dma_start(wv[:], wv_d[:])
            wo = cp.tile([128, KC, D], BF); nc.sync.dma_start(wo[:], wo_d[:])
            w1t = cp.tile([128, KC, FF], BF); nc.sync.dma_start(w1t[:], w1_d[:])
            w2t = cp.tile([128, FC, D], BF); nc.sync.dma_start(w2t[:], w2_d[:])
            qkb = cp.tile([128, 8], F32); nc.sync.dma_start(qkb[:], qkb_d[:])
            b1t = cp.tile([128, FC], F32); nc.sync.dma_start(b1t[:], b1_d[:])
            obc = cp.tile([128, D], F32); nc.sync.dma_start(obc[:], ob_d[:])
            b2r = cp.tile([1, D], BF); nc.sync.dma_start(b2r[:], b2_d[:])
            g1b = cp.tile([128, D], F32); nc.sync.dma_start(g1b[:], g1_d[:])
            g2b = cp.tile([128, D], F32); nc.sync.dma_start(g2b[:], g2_d[:])
            bb2 = cp.tile([128, D], F32); nc.sync.dma_start(bb2[:], bb2_d[:])
            ones1 = cp.tile([1, 128], BF); nc.sync.dma_start(ones1[:], on_d[:])
            idb = cp.tile([128, 128], BF); nc.sync.dma_start(idb[:], idb_d[:])
            eps_t = cp.tile([128, 1], F32); nc.vector.memset(eps_t[:], EPS)

            invD2 = 1.0 / (D * D)
            I32 = mybir.dt.int32
            magic = cp.tile([128, G], I32)
            nc.vector.memset(magic[:], 0x5F3759DF)

            def batched_rstd(st):
                """st [128, 4*G] f32: cols 0:G=sum(y), G:2G=sum(y^2) per
                window.  Writes cols 2G:3G = rstd = 1/sqrt(var+eps) (DVE
                bit-trick seed + 2 Newton steps; no ACT transcendentals)
                and cols 3G:4G = -mu*rstd."""
                sy, sq = st[:, 0:G], st[:, G:2 * G]
                rstd, nmr = st[:, 2 * G:3 * G], st[:, 3 * G:4 * G]
                a = sp.tile([128, G], F32, tag="lna")
                b = sp.tile([128, G], F32, tag="lnb")
                v = sp.tile([128, G], F32, tag="lnv")
                y0 = sp.tile([128, G], F32, tag="lny0")
                t = sp.tile([128, G], F32, tag="lnt1")
                nc.vector.tensor_mul(a[:], sy, sy)
                nc.vector.scalar_tensor_tensor(b[:], sq, float(D), a[:],
                                               ALU.mult, ALU.subtract)
                nc.vector.tensor_scalar(v[:], b[:], invD2, EPS,
                                        ALU.mult, ALU.add)
                vi = v[:].bitcast(I32)
                nc.vector.tensor_scalar(y0[:].bitcast(I32), vi, 1, None,
                                        ALU.arith_shift_right)
                nc.vector.scalar_tensor_tensor(y0[:].bitcast(I32), magic[:],
                                               0, y0[:].bitcast(I32),
                                               ALU.add, ALU.subtract)
                for _ in range(2):
                    nc.vector.tensor_mul(t[:], y0[:], y0[:])
                    nc.vector.tensor_mul(t[:], t[:], v[:])
                    nc.vector.tensor_scalar(t[:], t[:], -0.5, 1.5,
                                            ALU.mult, ALU.add)
                    nc.vector.tensor_mul(y0[:], y0[:], t[:])
                nc.vector.tensor_copy(rstd, y0[:])
                nc.vector.scalar_tensor_tensor(nmr, sy, -1.0 / D, rstd,
                                               ALU.mult, ALU.mult)

            def emit_ff(it, zt, zw):
                """Generator: FF1 + FF2 + LN2 + store for iteration `it`.
                Yields between small instruction groups so the caller can
                interleave them into the next iteration's attention phase
                (fills TensorE stalls so the HAM clock stays warm)."""
                h1 = sp.tile([128, FC, G * W], BF, tag="h1", name="h1")
                for m in range(FC):
                    ph = pbig.tile([128, G * W], F32, tag="big", name="ph")
                    for k in range(KC):
                        nc.tensor.matmul(
                            ph[:], w1t[:, k, m * 128:(m + 1) * 128],
                            zt[:, k, :], start=(k == 0), stop=(k == KC - 1))
                    nc.scalar.activation(h1[:, m, :], ph[:], AF.Relu,
                                         bias=b1t[:, m:m + 1])
                    yield
                st2 = sp.tile([128, 4 * G], F32, tag="st2", name="st2")
                y2w = []
                for w in range(G):
                    pf = pbig.tile([128, D], F32, tag="big", name="pf")
                    for m0 in range(0, FC, 4):
                        for m in range(m0, m0 + 4):
                            nc.tensor.matmul(
                                pf[:], h1[:, m, w * W:(w + 1) * W],
                                w2t[:, m, :], start=(m == 0), stop=False)
                        yield
                    nc.tensor.matmul(pf[:], ones1[:], b2r[:],
                                     start=False, stop=True)
                    y2g = sp.tile([128, D], F32, tag="y2g", name="y2g")
                    nc.vector.tensor_mul(y2g[:], zw[w][:], g1b[:])
                    y2 = sp.tile([128, D], F32, tag=f"y2_{w}",
                                 name=f"y2_{w}", bufs=1)
                    nc.vector.scalar_tensor_tensor(y2[:], pf[:], 0.0, y2g[:],
                                                   ALU.add, ALU.add,
                                                   accum_out=st2[:, w:w + 1])
                    sq2 = sp.tile([128, D], BF, tag="sq2", name="sq2")
                    nc.vector.scalar_tensor_tensor(sq2[:], y2[:], 0.0, y2[:],
                                                   ALU.add, ALU.mult,
                                                   accum_out=st2[:, G + w:G + w + 1])
                    y2w.append(y2)
                    yield
                batched_rstd(st2)
                yield
                for w in range(G):
                    z2 = sp.tile([128, D], F32, tag="z2", name="z2")
                    nc.scalar.activation(z2[:], y2w[w][:], AF.Identity,
                                         bias=st2[:, 3 * G + w:3 * G + w + 1],
                                         scale=st2[:, 2 * G + w:2 * G + w + 1])
                    yo = sp.tile([128, D], F32, tag="yo", name="yo")
                    nc.vector.tensor_mul(z2[:], z2[:], g2b[:])
                    nc.vector.tensor_add(yo[:], z2[:], bb2[:])
                    nc.sync.dma_start(ov[G * it + w], yo[:])
                    yield

            def drive(gen, n):
                if gen is None:
                    return
                for _ in range(n):
                    try:
                        next(gen)
                    except StopIteration:
                        return

            ffg = None
            for it in range(n_iters):
                # ---- loads + x transposes (f32 stationary, bf16 out) ----
                if it == 0:
                    x = x0
                else:
                    x = sp.tile([128, G, D], F32, tag="x")
                    for w in range(G):
                        nc.sync.dma_start(x[:, w, :], xv[G * it + w])
                xtp = sp.tile([128, KC, G * W], BF, tag="xtp")
                for w in range(G):
                    tpx = ptp.tile([128, KC, 128], F32, tag="tp")
                    for k in range(KC):
                        nc.tensor.transpose(tpx[:, k, :],
                                            x[:, w, k * 128:(k + 1) * 128],
                                            idf[:])
                    nc.vector.tensor_copy(xtp[:, :, w * W:(w + 1) * W], tpx[:])

                # ---- qkT (e-major) ----
                qkt = sp.tile([128, 8, G * W], BF, tag="qkt")
                for m in range(8):
                    pq = pbig.tile([128, G * W], F32, tag="big", name="pq")
                    for k in range(KC):
                        nc.tensor.matmul(
                            pq[:], wqk[:, k, m * 128:(m + 1) * 128],
                            xtp[:, k, :], start=(k == 0), stop=(k == KC - 1))
                    nc.scalar.activation(qkt[:, m, :], pq[:], AF.Identity,
                                         bias=qkb[:, m:m + 1])

                # ---- v token-major, with ones column per head ----
                v = sp.tile([128, G, H, VP], BF, tag="v")
                for w in range(G):
                    pv = pbig.tile([128, D], F32, tag="big", name="pv")
                    for k in range(KC):
                        nc.tensor.matmul(
                            pv[:], xtp[:, k, w * W:(w + 1) * W], wv[:, k, :],
                            start=(k == 0), stop=(k == KC - 1))
                    pvv = pv[:].rearrange("p (h e) -> p h e", h=H)
                    nc.vector.tensor_copy(v[:, w, :, 0:HD], pvv)
                    nc.vector.memset(v[:, w, :, HD:HD + 1], 1.0)

                # ---- scores (transposed), exp, attention + rowsum ----
                attok = [sp.tile([128, D], BF, tag=f"attok{w}", name=f"attok{w}")
                         for w in range(G)]
                for h in range(H):
                    j, pb = h // 2, (h % 2) * 64
                    sc = psc.tile([128, G * W], F32, tag="sc")
                    for w in range(G):
                        lk = qkt[pb:pb + 64, 4 + j, w * W:(w + 1) * W]
                        lq = qkt[pb:pb + 64, j, w * W:(w + 1) * W]
                        nc.tensor.matmul(sc[:, w * W:(w + 1) * W], lk, lq,
                                         start=True, stop=True,
                                         tile_position=(pb, 0))
                    pr = sp3.tile([128, G * W], BF, tag="pr")
                    nc.scalar.activation(pr[:], sc[:], AF.Exp)
                    drive(ffg, 1)
                    pat = ppat.tile([128, G, HD + 1], F32, tag="pat")
                    for w in range(G):
                        nc.tensor.matmul(
                            pat[:, w, :], pr[:, w * W:(w + 1) * W],
                            v[:, w, h, 0:HD + 1], start=True, stop=True)
                    rt = sp.tile([128, G, 1], F32, tag="rt")
                    nc.vector.reciprocal(rt[:], pat[:, :, HD:HD + 1])
                    for w in range(G):
                        nc.vector.tensor_scalar(
                            attok[w][:, h * HD:(h + 1) * HD],
                            pat[:, w, 0:HD], rt[:, w, :], None,
                            ALU.mult)
                    drive(ffg, 1)

                # ---- attn transpose + out-proj + residual ----
                zt = sp.tile([128, KC, G * W], BF, tag="zt")
                zw = [sp.tile([128, D], BF, tag=f"z{w}", name=f"z{w}")
                      for w in range(G)]
                st1 = sp.tile([128, 4 * G], F32, tag="st1")
                y1w = []
                for w in range(G):
                    tpa = ptp.tile([128, KC, 128], BF, tag="tp")
                    for k in range(KC):
                        nc.tensor.transpose(tpa[:, k, :],
                                            attok[w][:, k * 128:(k + 1) * 128],
                                            idb[:])
                    ats = sp.tile([128, KC, 128], BF, tag="ats")
                    nc.vector.tensor_copy(ats[:], tpa[:])
                    drive(ffg, 1)

                    pao = pbig.tile([128, D], F32, tag="big", name="pao")
                    for k in range(KC):
                        nc.tensor.matmul(pao[:], ats[:, k, :], wo[:, k, :],
                                         start=(k == 0), stop=(k == KC - 1))

                    xob = sp.tile([128, D], F32, tag="xob")
                    nc.vector.tensor_add(xob[:], x[:, w, :], obc[:])
                    y1 = sp.tile([128, D], F32, tag=f"y1_{w}",
                                 name=f"y1_{w}", bufs=1)
                    nc.vector.scalar_tensor_tensor(y1[:], pao[:], 0.0,
                                                   xob[:], ALU.add,
                                                   ALU.add,
                                                   accum_out=st1[:, w:w + 1])
                    sq = sp.tile([128, D], BF, tag="sq")
                    nc.vector.scalar_tensor_tensor(sq[:], y1[:], 0.0, y1[:],
                                                   ALU.add, ALU.mult,
                                                   accum_out=st1[:, G + w:G + w + 1])
                    y1w.append(y1)
                    drive(ffg, 2)

                # ---- LN1 + z transposes ----
                batched_rstd(st1)
                for w in range(G):
                    nc.scalar.activation(zw[w][:], y1w[w][:], AF.Identity,
                                         bias=st1[:, 3 * G + w:3 * G + w + 1],
                                         scale=st1[:, 2 * G + w:2 * G + w + 1])
                    tpz = ptp.tile([128, KC, 128], BF, tag="tp")
                    for k in range(KC):
                        nc.tensor.transpose(tpz[:, k, :],
                                            zw[w][:, k * 128:(k + 1) * 128],
                                            idb[:])
                    nc.vector.tensor_copy(zt[:, :, w * W:(w + 1) * W], tpz[:])
                    drive(ffg, 3)
                drive(ffg, 999)
                ffg = emit_ff(it, zt, zw)
            drive(ffg, 999)

    nc.compile()
    return nc


def _pack(wT, kc):
    """[kc*128, N] -> [128, kc*N] with partition p, block k = wT[k*128+p]."""
    n = wT.shape[1]
    return np.ascontiguousarray(
        wT.reshape(kc, 128, n).transpose(1, 0, 2).reshape(128, kc * n))


_CACHE = {}


def _get_nc(n_iters=ITERS):
    if n_iters not in _CACHE:
        _CACHE[n_iters] = _build_nc(n_iters)
    return _CACHE[n_iters]


def _prep_inputs(src, in_proj_w, in_proj_b, out_w, out_b, ln1_g, ln1_b,
                 w1, b1, w2, b2, ln2_g, ln2_b, n_iters=ITERS):
    src = np.asarray(src, np.float32)
    scale = 1.0 / np.sqrt(HD)

    in_proj_w = np.asarray(in_proj_w, np.float32)
    in_proj_b = np.asarray(in_proj_b, np.float32)
    out_w = np.asarray(out_w, np.float32)
    out_b = np.asarray(out_b, np.float32)
    w1 = np.asarray(w1, np.float32)
    b1 = np.asarray(b1, np.float32)
    w2 = np.asarray(w2, np.float32)
    b2 = np.asarray(b2, np.float32)
    ln1_g = np.asarray(ln1_g, np.float32)
    ln1_b = np.asarray(ln1_b, np.float32)
    ln2_g = np.asarray(ln2_g, np.float32)
    ln2_b = np.asarray(ln2_b, np.float32)

    # q scaled by 1/sqrt(HD); k-bias is softmax-invariant (constant along
    # the key axis for fixed q) and dropped.
    wqkT = in_proj_w[:2 * D].T.copy()       # [512, 1024], q cols then k cols
    wqkT[:, :D] *= scale
    bqk = np.zeros((2 * D,), np.float32)
    bqk[:D] = in_proj_b[:D] * scale

    bv = in_proj_b[2 * D:]
    obp = out_b + out_w @ bv                # fold v-bias through out proj
    w1f = w1 * ln1_g[None, :]               # fold LN1 gain into FF1
    b1f = b1 + w1 @ ln1_b                   # fold LN1 bias into FF1 bias
    b2f = b2 + ln1_b                        # fold LN1 bias into residual-2

    common = {
        "wqk": _pack(wqkT.astype(BF16), KC),
        "wv": _pack(in_proj_w[2 * D:].T.astype(BF16), KC),
        "wo": _pack(out_w.T.astype(BF16), KC),
        "w1t": _pack(w1f.T.astype(BF16), KC),
        "w2t": _pack(w2.T.astype(BF16), FC),
        "qkb": np.ascontiguousarray(bqk.reshape(8, 128).T),
        "b1t": np.ascontiguousarray(b1f.reshape(FC, 128).T),
        "obc": np.ascontiguousarray(np.broadcast_to(obp, (128, D))),
        "b2r": b2f.astype(BF16)[None, :],
        "g1b": np.ascontiguousarray(np.broadcast_to(ln1_g, (128, D))),
        "g2b": np.ascontiguousarray(np.broadcast_to(ln2_g, (128, D))),
        "bb2": np.ascontiguousarray(np.broadcast_to(ln2_b, (128, D))),
        "ones1": np.ones((1, 128), BF16),
        "idb": np.eye(128, dtype=BF16),
        "idf": np.eye(128, dtype=np.float32),
    }

    wins = src.reshape(NW_TOT, W, D)
    wpc = n_iters * G
    in_maps = []
    for c in range(N_CORES):
        m = dict(common)
        m["x"] = np.ascontiguousarray(
            wins[c * wpc:(c + 1) * wpc].reshape(wpc * W, D))
        in_maps.append(m)
    return in_maps


def kernel(src, in_proj_w, in_proj_b, out_w, out_b, ln1_g, ln1_b,
           w1, b1, w2, b2, ln2_g, ln2_b):
    nc = _get_nc()
    in_maps = _prep_inputs(src, in_proj_w, in_proj_b, out_w, out_b, ln1_g,
                           ln1_b, w1, b1, w2, b2, ln2_g, ln2_b)
    res = run_bass_kernel_spmd(nc, in_maps, list(range(N_CORES)))
    out = np.concatenate([res.results[c]["out"] for c in range(N_CORES)], axis=0)
    return np.ascontiguousarray(out.reshape(B, S, D)).astype(np.float32)


# revision 19
# speedup vs baseline: 1.1560x; 1.1560x over previous
"""Local-window attention encoder layer on 8 Trainium2 cores.

Problem: B=4, S=8192, D=512, window W=128, H=8 heads (HD=64), FF dim 2048.
Sharding: [B*nW]=256 independent windows split 32/core across 8 cores.

Per-core device kernel, restructured for engine balance (v2):
  4 windows (512 tokens) per iteration so every big matmul streams a
  512-wide moving operand.  Scores are produced TRANSPOSED ([k,q]) so
  softmax exp needs no per-row accumulation: exp runs as one batched
  ACT op per head over all 4 windows, and the softmax denominator rides
  the attention matmul itself via a ones-column appended to V (the
  probs stationary contracts over k, so an extra moving column of ones
  yields sum_k p[k,q] in the same instruction).  Normalization is a
  per-partition divide during the PSUM->SBUF read of the token-major
  attention output.  LayerNorm rstd uses exp(-0.5*ln(var+eps)) so the
  ACT engine stays on the natural_log_exp table set for the whole
  kernel (no table reloads); FF1's relu+bias also runs on ACT (relu is
  in every set).  LN gains/biases are folded host-side where linear
  algebra allows (g1 into W1, b1+=W1@ln1_b, b2+=ln1_b, out_b+=Wo@bv).
"""

import numpy as np
import ml_dtypes

import concourse.bass as bass
import concourse.tile as tile
from concourse import bacc, mybir
from concourse.bass_utils import run_bass_kernel_spmd

BF16 = ml_dtypes.bfloat16
F32 = mybir.dt.float32
BF = mybir.dt.bfloat16
AF = mybir.ActivationFunctionType
ALU = mybir.AluOpType

D = 512
H = 8
W = 128
HD = 64
FF = 2048
EPS = 1e-5
N_CORES = 8
B, S = 4, 8192
NW_TOT = (B * S) // W          # 256 windows
WPC = NW_TOT // N_CORES        # 32 windows per core
G = 4                          # windows per iteration
ITERS = WPC // G               # 8 iterations
KC = D // 128                  # 4 contraction chunks of 128
FC = FF // 128                 # 16 ff chunks
VP = 68                        # v row pitch (64 chans + ones col + pad)


def _build_nc(n_iters=ITERS):
    nc = bacc.Bacc("TRN2", target_bir_lowering=False, debug=False,
                   num_devices=N_CORES)
    n_tok = n_iters * G * W

    x_d = nc.dram_tensor("x", [n_tok, D], F32, kind="ExternalInput").ap()
    out_d = nc.dram_tensor("out", [n_tok, D], F32, kind="ExternalOutput").ap()
    wqk_d = nc.dram_tensor("wqk", [128, KC * 1024], BF, kind="ExternalInput").ap()
    wv_d = nc.dram_tensor("wv", [128, KC * D], BF, kind="ExternalInput").ap()
    wo_d = nc.dram_tensor("wo", [128, KC * D], BF, kind="ExternalInput").ap()
    w1_d = nc.dram_tensor("w1t", [128, KC * FF], BF, kind="ExternalInput").ap()
    w2_d = nc.dram_tensor("w2t", [128, FC * D], BF, kind="ExternalInput").ap()
    qkb_d = nc.dram_tensor("qkb", [128, 8], F32, kind="ExternalInput").ap()
    b1_d = nc.dram_tensor("b1t", [128, FC], F32, kind="ExternalInput").ap()
    ob_d = nc.dram_tensor("obc", [128, D], F32, kind="ExternalInput").ap()
    b2_d = nc.dram_tensor("b2r", [1, D], BF, kind="ExternalInput").ap()
    g1_d = nc.dram_tensor("g1b", [128, D], F32, kind="ExternalInput").ap()
    g2_d = nc.dram_tensor("g2b", [128, D], F32, kind="ExternalInput").ap()
    bb2_d = nc.dram_tensor("bb2", [128, D], F32, kind="ExternalInput").ap()
    on_d = nc.dram_tensor("ones1", [1, 128], BF, kind="ExternalInput").ap()
    idb_d = nc.dram_tensor("idb", [128, 128], BF, kind="ExternalInput").ap()
    idf_d = nc.dram_tensor("idf", [128, 128], F32, kind="ExternalInput").ap()

    xv = x_d.rearrange("(w p) d -> w p d", p=W)
    ov = out_d.rearrange("(w p) d -> w p d", p=W)

    with tile.TileContext(nc) as tc:
        with (
            tc.tile_pool(name="const", bufs=1) as cp,
            tc.tile_pool(name="stream", bufs=2) as sp,
            tc.tile_pool(name="pr3", bufs=3) as sp3,
            tc.tile_pool(name="big", bufs=4, space="PSUM") as pbig,
            tc.tile_pool(name="sc", bufs=1, space="PSUM") as psc,
            tc.tile_pool(name="pat", bufs=1, space="PSUM") as ppat,
            tc.tile_pool(name="tp", bufs=2, space="PSUM") as ptp,
        ):
            # ---- resident constants ----
            # first iteration's activations + small consts first so the PE
            # can start transposing while the big weights stream in
            idf = cp.tile([128, 128], F32); nc.sync.dma_start(idf[:], idf_d[:])
            x0 = sp.tile([128, G, D], F32, tag="x", name="x0")
            for w0_ in range(G):
                nc.sync.dma_start(x0[:, w0_, :], xv[w0_])
            wqk = cp.tile([128, KC, 1024], BF); nc.sync.dma_start(wqk[:], wqk_d[:])
            wv = cp.tile([128, KC, D], BF); nc.sync.dma_start(wv[:], wv_d[:])
            wo = cp.tile([128, KC, D], BF); nc.sync.dma_start(wo[:], wo_d[:])
            w1t = cp.tile([128, KC, FF], BF); nc.sync.dma_start(w1t[:], w1_d[:])
            w2t = cp.tile([128, FC, D], BF); nc.sync.dma_start(w2t[:], w2_d[:])
            qkb = cp.tile([128, 8], F32); nc.sync.dma_start(qkb[:], qkb_d[:])
            b1t = cp.tile([128, FC], F32); nc.sync.dma_start(b1t[:], b1_d[:])
            obc = cp.tile([128, D], F32); nc.sync.dma_start(obc[:], ob_d[:])
            b2r = cp.tile([1, D], BF); nc.sync.dma_start(b2r[:], b2_d[:])
            g1b = cp.tile([128, D], F32); nc.sync.dma_start(g1b[:], g1_d[:])
            g2b = cp.tile([128, D], F32); nc.sync.dma_start(g2b[:], g2_d[:])
            bb2 = cp.tile([128, D], F32); nc.sync.dma_start(bb2[:], bb2_d[:])
            ones1 = cp.tile([1, 128], BF); nc.sync.dma_start(ones1[:], on_d[:])
            idb = cp.tile([128, 128], BF); nc.sync.dma_start(idb[:], idb_d[:])
            eps_t = cp.tile([128, 1], F32); nc.vector.memset(eps_t[:], EPS)

            invD2 = 1.0 / (D * D)
            I32 = mybir.dt.int32
            magic = cp.tile([128, G], I32)
            nc.vector.memset(magic[:], 0x5F3759DF)

            def batched_rstd(st):
                """st [128, 4*G] f32: cols 0:G=sum(y), G:2G=sum(y^2) per
                window.  Writes cols 2G:3G = rstd = 1/sqrt(var+eps) (DVE
                bit-trick seed + 2 Newton steps; no ACT transcendentals)
                and cols 3G:4G = -mu*rstd."""
                sy, sq = st[:, 0:G], st[:, G:2 * G]
                rstd, nmr = st[:, 2 * G:3 * G], st[:, 3 * G:4 * G]
                a = sp.tile([128, G], F32, tag="lna")
                b = sp.tile([128, G], F32, tag="lnb")
                v = sp.tile([128, G], F32, tag="lnv")
                y0 = sp.tile([128, G], F32, tag="lny0")
                t = sp.tile([128, G], F32, tag="lnt1")
                nc.vector.tensor_mul(a[:], sy, sy)
                nc.vector.scalar_tensor_tensor(b[:], sq, float(D), a[:],
                                               ALU.mult, ALU.subtract)
                nc.vector.tensor_scalar(v[:], b[:], invD2, EPS,
                                        ALU.mult, ALU.add)
                vi = v[:].bitcast(I32)
                nc.vector.tensor_scalar(y0[:].bitcast(I32), vi, 1, None,
                                        ALU.arith_shift_right)
                nc.vector.scalar_tensor_tensor(y0[:].bitcast(I32), magic[:],
                                               0, y0[:].bitcast(I32),
                                               ALU.add, ALU.subtract)
                for _ in range(2):
                    nc.vector.tensor_mul(t[:], y0[:], y0[:])
                    nc.vector.tensor_mul(t[:], t[:], v[:])
                    nc.vector.tensor_scalar(t[:], t[:], -0.5, 1.5,
                                            ALU.mult, ALU.add)
                    nc.vector.tensor_mul(y0[:], y0[:], t[:])
                nc.vector.tensor_copy(rstd, y0[:])
                nc.vector.scalar_tensor_tensor(nmr, sy, -1.0 / D, rstd,
                                               ALU.mult, ALU.mult)

            def emit_ff(it, zt, zw):
                """Generator: FF1 + FF2 + LN2 + store for iteration `it`.
                Yields between small instruction groups so the caller can
                interleave them into the next iteration's attention phase
                (fills TensorE stalls so the HAM clock stays warm)."""
                h1 = sp.tile([128, FC, G * W], BF, tag="h1", name="h1")
                for m in range(FC):
                    ph = pbig.tile([128, G * W], F32, tag="big", name="ph")
                    for k in range(KC):
                        nc.tensor.matmul(
                            ph[:], w1t[:, k, m * 128:(m + 1) * 128],
                            zt[:, k, :], start=(k == 0), stop=(k == KC - 1))
                    nc.scalar.activation(h1[:, m, :], ph[:], AF.Relu,
                                         bias=b1t[:, m:m + 1])
                    yield
                st2 = sp.tile([128, 4 * G], F32, tag="st2", name="st2")
                y2w = []
                for w in range(G):
                    pf = pbig.tile([128, D], F32, tag="big", name="pf")
                    for m0 in range(0, FC, 4):
                        for m in range(m0, m0 + 4):
                            nc.tensor.matmul(
                                pf[:], h1[:, m, w * W:(w + 1) * W],
                                w2t[:, m, :], start=(m == 0), stop=False)
                        yield
                    nc.tensor.matmul(pf[:], ones1[:], b2r[:],
                                     start=False, stop=True)
                    y2g = sp.tile([128, D], F32, tag="y2g", name="y2g")
                    nc.vector.tensor_mul(y2g[:], zw[w][:], g1b[:])
                    y2 = sp.tile([128, D], F32, tag=f"y2_{w}",
                                 name=f"y2_{w}", bufs=1)
                    nc.vector.scalar_tensor_tensor(y2[:], pf[:], 0.0, y2g[:],
                                                   ALU.add, ALU.add,
                                                   accum_out=st2[:, w:w + 1])
                    sq2 = sp.tile([128, D], BF, tag="sq2", name="sq2")
                    nc.vector.scalar_tensor_tensor(sq2[:], y2[:], 0.0, y2[:],
                                                   ALU.add, ALU.mult,
                                                   accum_out=st2[:, G + w:G + w + 1])
                    y2w.append(y2)
                    yield
                batched_rstd(st2)
                yield
                for w in range(G):
                    z2 = sp.tile([128, D], F32, tag="z2", name="z2")
                    nc.vector.tensor_scalar(z2[:], y2w[w][:],
                                            st2[:, 2 * G + w:2 * G + w + 1],
                                            st2[:, 3 * G + w:3 * G + w + 1],
                                            ALU.mult, ALU.add)
                    yo = sp.tile([128, D], F32, tag="yo", name="yo")
                    nc.vector.tensor_mul(z2[:], z2[:], g2b[:])
                    nc.vector.tensor_add(yo[:], z2[:], bb2[:])
                    nc.sync.dma_start(ov[G * it + w], yo[:])
                    yield

            def drive(gen, n):
                if gen is None:
                    return
                for _ in range(n):
                    try:
                        next(gen)
                    except StopIteration:
                        return

            ffg = None
            for it in range(n_iters):
                # ---- loads + x transposes (f32 stationary, bf16 out) ----
                if it == 0:
                    x = x0
                else:
                    x = sp.tile([128, G, D], F32, tag="x")
                    for w in range(G):
                        nc.sync.dma_start(x[:, w, :], xv[G * it + w])
                xtp = sp.tile([128, KC, G * W], BF, tag="xtp")
                for w in range(G):
                    tpx = ptp.tile([128, KC, 128], F32, tag="tp")
                    for k in range(KC):
                        nc.tensor.transpose(tpx[:, k, :],
                                            x[:, w, k * 128:(k + 1) * 128],
                                            idf[:])
                    nc.vector.tensor_copy(xtp[:, :, w * W:(w + 1) * W], tpx[:])

                # ---- qkT (e-major) ----
                qkt = sp.tile([128, 8, G * W], BF, tag="qkt")
                for m in range(8):
                    pq = pbig.tile([128, G * W], F32, tag="big", name="pq")
                    for k in range(KC):
                        nc.tensor.matmul(
                            pq[:], wqk[:, k, m * 128:(m + 1) * 128],
                            xtp[:, k, :], start=(k == 0), stop=(k == KC - 1))
                    nc.scalar.activation(qkt[:, m, :], pq[:], AF.Identity,
                                         bias=qkb[:, m:m + 1])

                # ---- v token-major, with ones column per head ----
                v = sp.tile([128, G, H, VP], BF, tag="v")
                for w in range(G):
                    pv = pbig.tile([128, D], F32, tag="big", name="pv")
                    for k in range(KC):
                        nc.tensor.matmul(
                            pv[:], xtp[:, k, w * W:(w + 1) * W], wv[:, k, :],
                            start=(k == 0), stop=(k == KC - 1))
                    pvv = pv[:].rearrange("p (h e) -> p h e", h=H)
                    nc.vector.tensor_copy(v[:, w, :, 0:HD], pvv)
                    nc.vector.memset(v[:, w, :, HD:HD + 1], 1.0)

                # ---- scores (transposed), exp, attention + rowsum ----
                attok = [sp.tile([128, D], BF, tag=f"attok{w}", name=f"attok{w}")
                         for w in range(G)]
                for h in range(H):
                    j, pb = h // 2, (h % 2) * 64
                    sc = psc.tile([128, G * W], F32, tag="sc")
                    for w in range(G):
                        lk = qkt[pb:pb + 64, 4 + j, w * W:(w + 1) * W]
                        lq = qkt[pb:pb + 64, j, w * W:(w + 1) * W]
                        nc.tensor.matmul(sc[:, w * W:(w + 1) * W], lk, lq,
                                         start=True, stop=True,
                                         tile_position=(pb, 0))
                    pr = sp3.tile([128, G * W], BF, tag="pr")
                    nc.scalar.activation(pr[:], sc[:], AF.Exp)
                    drive(ffg, 1)
                    pat = ppat.tile([128, G, HD + 1], F32, tag="pat")
                    for w in range(G):
                        nc.tensor.matmul(
                            pat[:, w, :], pr[:, w * W:(w + 1) * W],
                            v[:, w, h, 0:HD + 1], start=True, stop=True)
                    rt = sp.tile([128, G, 1], F32, tag="rt")
                    nc.vector.reciprocal(rt[:], pat[:, :, HD:HD + 1])
                    for w in range(G):
                        nc.vector.tensor_scalar(
                            attok[w][:, h * HD:(h + 1) * HD],
                            pat[:, w, 0:HD], rt[:, w, :], None,
                            ALU.mult)
                    drive(ffg, 1)

                # ---- attn transpose + out-proj + residual ----
                zt = sp.tile([128, KC, G * W], BF, tag="zt")
                zw = [sp.tile([128, D], BF, tag=f"z{w}", name=f"z{w}")
                      for w in range(G)]
                st1 = sp.tile([128, 4 * G], F32, tag="st1")
                y1w = []
                for w in range(G):
                    tpa = ptp.tile([128, KC, 128], BF, tag="tp")
                    for k in range(KC):
                        nc.tensor.transpose(tpa[:, k, :],
                                            attok[w][:, k * 128:(k + 1) * 128],
                                            idb[:])
                    ats = sp.tile([128, KC, 128], BF, tag="ats")
                    nc.vector.tensor_copy(ats[:], tpa[:])
                    drive(ffg, 1)

                    pao = pbig.tile([128, D], F32, tag="big", name="pao")
                    for k in range(KC):
                        nc.tensor.matmul(pao[:], ats[:, k, :], wo[:, k, :],
                                         start=(k == 0), stop=(k == KC - 1))

                    xob = sp.tile([128, D], F32, tag="xob")
                    nc.vector.tensor_add(xob[:], x[:, w, :], obc[:])
                    y1 = sp.tile([128, D], F32, tag=f"y1_{w}",
                                 name=f"y1_{w}", bufs=1)
                    nc.vector.scalar_tensor_tensor(y1[:], pao[:], 0.0,
                                                   xob[:], ALU.add,
                                                   ALU.add,
                                                   accum_out=st1[:, w:w + 1])
                    sq = sp.tile([128, D], BF, tag="sq")
                    nc.vector.scalar_tensor_tensor(sq[:], y1[:], 0.0, y1[:],
                                                   ALU.add, ALU.mult,
                                                   accum_out=st1[:, G + w:G + w + 1])
                    y1w.append(y1)
                    drive(ffg, 2)

                # ---- LN1 + z transposes ----
                batched_rstd(st1)
                for w in range(G):
                    nc.vector.tensor_scalar(zw[w][:], y1w[w][:],
                                            st1[:, 2 * G + w:2 * G + w + 1],
                                            st1[:, 3 * G + w:3 * G + w + 1],
                                            ALU.mult, ALU.add)
                    tpz = ptp.tile([128, KC, 128], BF, tag="tp")
                    for k in range(KC):
                        nc.tensor.transpose(tpz[:, k, :],
                                            zw[w][:, k * 128:(k + 1) * 128],
                                            idb[:])
                    nc.vector.tensor_copy(zt[:, :, w * W:(w + 1) * W], tpz[:])
                    drive(ffg, 3)
                drive(ffg, 999)
                ffg = emit_ff(it, zt, zw)
            drive(ffg, 999)

    nc.compile()
    return nc


def _pack(wT, kc):
    """[kc*128, N] -> [128, kc*N] with partition p, block k = wT[k*128+p]."""
    n = wT.shape[1]
    return np.ascontiguousarray(
        wT.reshape(kc, 128, n).transpose(1, 0, 2).reshape(128, kc * n))


_CACHE = {}


def _get_nc(n_iters=ITERS):
    if n_iters not in _CACHE:
        _CACHE[n_iters] = _build_nc(n_iters)
    return _CACHE[n_iters]


def _prep_inputs(src, in_proj_w, in_proj_b, out_w, out_b, ln1_g, ln1_b,
                 w1, b1, w2, b2, ln2_g, ln2_b, n_iters=ITERS):
    src = np.asarray(src, np.float32)
    scale = 1.0 / np.sqrt(HD)

    in_proj_w = np.asarray(in_proj_w, np.float32)
    in_proj_b = np.asarray(in_proj_b, np.float32)
    out_w = np.asarray(out_w, np.float32)
    out_b = np.asarray(out_b, np.float32)
    w1 = np.asarray(w1, np.float32)
    b1 = np.asarray(b1, np.float32)
    w2 = np.asarray(w2, np.float32)
    b2 = np.asarray(b2, np.float32)
    ln1_g = np.asarray(ln1_g, np.float32)
    ln1_b = np.asarray(ln1_b, np.float32)
    ln2_g = np.asarray(ln2_g, np.float32)
    ln2_b = np.asarray(ln2_b, np.float32)

    # q scaled by 1/sqrt(HD); k-bias is softmax-invariant (constant along
    # the key axis for fixed q) and dropped.
    wqkT = in_proj_w[:2 * D].T.copy()       # [512, 1024], q cols then k cols
    wqkT[:, :D] *= scale
    bqk = np.zeros((2 * D,), np.float32)
    bqk[:D] = in_proj_b[:D] * scale

    bv = in_proj_b[2 * D:]
    obp = out_b + out_w @ bv                # fold v-bias through out proj
    w1f = w1 * ln1_g[None, :]               # fold LN1 gain into FF1
    b1f = b1 + w1 @ ln1_b                   # fold LN1 bias into FF1 bias
    b2f = b2 + ln1_b                        # fold LN1 bias into residual-2

    common = {
        "wqk": _pack(wqkT.astype(BF16), KC),
        "wv": _pack(in_proj_w[2 * D:].T.astype(BF16), KC),
        "wo": _pack(out_w.T.astype(BF16), KC),
        "w1t": _pack(w1f.T.astype(BF16), KC),
        "w2t": _pack(w2.T.astype(BF16), FC),
        "qkb": np.ascontiguousarray(bqk.reshape(8, 128).T),
        "b1t": np.ascontiguousarray(b1f.reshape(FC, 128).T),
        "obc": np.ascontiguousarray(np.broadcast_to(obp, (128, D))),
        "b2r": b2f.astype(BF16)[None, :],
        "g1b": np.ascontiguousarray(np.broadcast_to(ln1_g, (128, D))),
        "g2b": np.ascontiguousarray(np.broadcast_to(ln2_g, (128, D))),
        "bb2": np.ascontiguousarray(np.broadcast_to(ln2_b, (128, D))),
        "ones1": np.ones((1, 128), BF16),
        "idb": np.eye(128, dtype=BF16),
        "idf": np.eye(128, dtype=np.float32),
    }

    wins = src.reshape(NW_TOT, W, D)
    wpc = n_iters * G
    in_maps = []
    for c in range(N_CORES):
        m = dict(common)
        m["x"] = np.ascontiguousarray(
            wins[c * wpc:(c + 1) * wpc].reshape(wpc * W, D))
        in_maps.append(m)
    return in_maps


def kernel(src, in_proj_w, in_proj_b, out_w, out_b, ln1_g, ln1_b,
           w1, b1, w2, b2, ln2_g, ln2_b):
    nc = _get_nc()
    in_maps = _prep_inputs(src, in_proj_w, in_proj_b, out_w, out_b, ln1_g,
                           ln1_b, w1, b1, w2, b2, ln2_g, ln2_b)
    res = run_bass_kernel_spmd(nc, in_maps, list(range(N_CORES)))
    out = np.concatenate([res.results[c]["out"] for c in range(N_CORES)], axis=0)
    return np.ascontiguousarray(out.reshape(B, S, D)).astype(np.float32)


# revision 21
# speedup vs baseline: 1.2158x; 1.0517x over previous
"""Local-window attention encoder layer on 8 Trainium2 cores.

Problem: B=4, S=8192, D=512, window W=128, H=8 heads (HD=64), FF dim 2048.
Sharding: [B*nW]=256 independent windows split 32/core across 8 cores.

Per-core device kernel, restructured for engine balance (v2):
  4 windows (512 tokens) per iteration so every big matmul streams a
  512-wide moving operand.  Scores are produced TRANSPOSED ([k,q]) so
  softmax exp needs no per-row accumulation: exp runs as one batched
  ACT op per head over all 4 windows, and the softmax denominator rides
  the attention matmul itself via a ones-column appended to V (the
  probs stationary contracts over k, so an extra moving column of ones
  yields sum_k p[k,q] in the same instruction).  Normalization is a
  per-partition divide during the PSUM->SBUF read of the token-major
  attention output.  LayerNorm rstd uses exp(-0.5*ln(var+eps)) so the
  ACT engine stays on the natural_log_exp table set for the whole
  kernel (no table reloads); FF1's relu+bias also runs on ACT (relu is
  in every set).  LN gains/biases are folded host-side where linear
  algebra allows (g1 into W1, b1+=W1@ln1_b, b2+=ln1_b, out_b+=Wo@bv).
"""

import numpy as np
import ml_dtypes

import concourse.bass as bass
import concourse.tile as tile
from concourse import bacc, mybir
from concourse.bass_utils import run_bass_kernel_spmd

BF16 = ml_dtypes.bfloat16
F32 = mybir.dt.float32
BF = mybir.dt.bfloat16
AF = mybir.ActivationFunctionType
ALU = mybir.AluOpType

D = 512
H = 8
W = 128
HD = 64
FF = 2048
EPS = 1e-5
N_CORES = 8
B, S = 4, 8192
NW_TOT = (B * S) // W          # 256 windows
WPC = NW_TOT // N_CORES        # 32 windows per core
G = 4                          # windows per iteration
ITERS = WPC // G               # 8 iterations
KC = D // 128                  # 4 contraction chunks of 128
FC = FF // 128                 # 16 ff chunks
VP = 68                        # v row pitch (64 chans + ones col + pad)


def _build_nc(n_iters=ITERS):
    nc = bacc.Bacc("TRN2", target_bir_lowering=False, debug=False,
                   num_devices=N_CORES)
    n_tok = n_iters * G * W

    x_d = nc.dram_tensor("x", [n_tok, D], F32, kind="ExternalInput").ap()
    out_d = nc.dram_tensor("out", [n_tok, D], F32, kind="ExternalOutput").ap()
    wqk_d = nc.dram_tensor("wqk", [128, KC * 1024], BF, kind="ExternalInput").ap()
    wv_d = nc.dram_tensor("wv", [128, KC * D], BF, kind="ExternalInput").ap()
    wo_d = nc.dram_tensor("wo", [128, KC * D], BF, kind="ExternalInput").ap()
    w1_d = nc.dram_tensor("w1t", [128, KC * FF], BF, kind="ExternalInput").ap()
    w2_d = nc.dram_tensor("w2t", [128, FC * D], BF, kind="ExternalInput").ap()
    qkb_d = nc.dram_tensor("qkb", [128, 8], F32, kind="ExternalInput").ap()
    b1_d = nc.dram_tensor("b1t", [128, FC], F32, kind="ExternalInput").ap()
    ob_d = nc.dram_tensor("obc", [128, D], F32, kind="ExternalInput").ap()
    b2_d = nc.dram_tensor("b2r", [1, D], BF, kind="ExternalInput").ap()
    g1_d = nc.dram_tensor("g1b", [128, D], F32, kind="ExternalInput").ap()
    g2_d = nc.dram_tensor("g2b", [128, D], F32, kind="ExternalInput").ap()
    bb2_d = nc.dram_tensor("bb2", [128, D], F32, kind="ExternalInput").ap()
    on_d = nc.dram_tensor("ones1", [1, 128], BF, kind="ExternalInput").ap()
    idb_d = nc.dram_tensor("idb", [128, 128], BF, kind="ExternalInput").ap()
    idf_d = nc.dram_tensor("idf", [128, 128], F32, kind="ExternalInput").ap()

    xv = x_d.rearrange("(w p) d -> w p d", p=W)
    ov = out_d.rearrange("(w p) d -> w p d", p=W)

    with tile.TileContext(nc) as tc:
        with (
            tc.tile_pool(name="const", bufs=1) as cp,
            tc.tile_pool(name="stream", bufs=2) as sp,
            tc.tile_pool(name="pr3", bufs=3) as sp3,
            tc.tile_pool(name="big", bufs=4, space="PSUM") as pbig,
            tc.tile_pool(name="sc", bufs=1, space="PSUM") as psc,
            tc.tile_pool(name="pat", bufs=1, space="PSUM") as ppat,
            tc.tile_pool(name="tp", bufs=2, space="PSUM") as ptp,
        ):
            # ---- resident constants ----
            # first iteration's activations + small consts first so the PE
            # can start transposing while the big weights stream in
            idf = cp.tile([128, 128], F32); nc.sync.dma_start(idf[:], idf_d[:])
            x0 = sp.tile([128, G, D], F32, tag="x", name="x0")
            for w0_ in range(G):
                nc.sync.dma_start(x0[:, w0_, :], xv[w0_])
            wqk = cp.tile([128, KC, 1024], BF); nc.sync.dma_start(wqk[:], wqk_d[:])
            wv = cp.tile([128, KC, D], BF); nc.sync.dma_start(wv[:], wv_d[:])
            wo = cp.tile([128, KC, D], BF); nc.sync.dma_start(wo[:], wo_d[:])
            qkb = cp.tile([128, 8], F32); nc.sync.dma_start(qkb[:], qkb_d[:])
            b1t = cp.tile([128, FC], F32); nc.sync.dma_start(b1t[:], b1_d[:])
            obc = cp.tile([128, D], F32); nc.sync.dma_start(obc[:], ob_d[:])
            b2r = cp.tile([1, D], BF); nc.sync.dma_start(b2r[:], b2_d[:])
            g1b = cp.tile([128, D], F32); nc.sync.dma_start(g1b[:], g1_d[:])
            g2b = cp.tile([128, D], F32); nc.sync.dma_start(g2b[:], g2_d[:])
            bb2 = cp.tile([128, D], F32); nc.sync.dma_start(bb2[:], bb2_d[:])
            ones1 = cp.tile([1, 128], BF); nc.sync.dma_start(ones1[:], on_d[:])
            idb = cp.tile([128, 128], BF); nc.sync.dma_start(idb[:], idb_d[:])
            eps_t = cp.tile([128, 1], F32); nc.vector.memset(eps_t[:], EPS)
            # FF weights last: not needed until the first FF phase (~60us in),
            # so they must not delay wqk/wv/wo which gate iteration 0
            w1t = cp.tile([128, KC, FF], BF); nc.sync.dma_start(w1t[:], w1_d[:])
            w2t = cp.tile([128, FC, D], BF); nc.sync.dma_start(w2t[:], w2_d[:])

            invD2 = 1.0 / (D * D)
            I32 = mybir.dt.int32
            magic = cp.tile([128, G], I32)
            nc.vector.memset(magic[:], 0x5F3759DF)

            def batched_rstd(st):
                """st [128, 4*G] f32: cols 0:G=sum(y), G:2G=sum(y^2) per
                window.  Writes cols 2G:3G = rstd = 1/sqrt(var+eps) (DVE
                bit-trick seed + 2 Newton steps; no ACT transcendentals)
                and cols 3G:4G = -mu*rstd."""
                sy, sq = st[:, 0:G], st[:, G:2 * G]
                rstd, nmr = st[:, 2 * G:3 * G], st[:, 3 * G:4 * G]
                a = sp.tile([128, G], F32, tag="lna")
                b = sp.tile([128, G], F32, tag="lnb")
                v = sp.tile([128, G], F32, tag="lnv")
                y0 = sp.tile([128, G], F32, tag="lny0")
                t = sp.tile([128, G], F32, tag="lnt1")
                nc.vector.tensor_mul(a[:], sy, sy)
                nc.vector.scalar_tensor_tensor(b[:], sq, float(D), a[:],
                                               ALU.mult, ALU.subtract)
                nc.vector.tensor_scalar(v[:], b[:], invD2, EPS,
                                        ALU.mult, ALU.add)
                vi = v[:].bitcast(I32)
                nc.vector.tensor_scalar(y0[:].bitcast(I32), vi, 1, None,
                                        ALU.arith_shift_right)
                nc.vector.scalar_tensor_tensor(y0[:].bitcast(I32), magic[:],
                                               0, y0[:].bitcast(I32),
                                               ALU.add, ALU.subtract)
                for _ in range(2):
                    nc.vector.tensor_mul(t[:], y0[:], y0[:])
                    nc.vector.tensor_mul(t[:], t[:], v[:])
                    nc.vector.tensor_scalar(t[:], t[:], -0.5, 1.5,
                                            ALU.mult, ALU.add)
                    nc.vector.tensor_mul(y0[:], y0[:], t[:])
                nc.vector.tensor_copy(rstd, y0[:])
                nc.vector.scalar_tensor_tensor(nmr, sy, -1.0 / D, rstd,
                                               ALU.mult, ALU.mult)

            def emit_ff(it, zt, zw):
                """Generator: FF1 + FF2 + LN2 + store for iteration `it`.
                Yields between small instruction groups so the caller can
                interleave them into the next iteration's attention phase
                (fills TensorE stalls so the HAM clock stays warm)."""
                h1 = sp.tile([128, FC, G * W], BF, tag="h1", name="h1")
                for m in range(FC):
                    ph = pbig.tile([128, G * W], F32, tag="big", name="ph")
                    for k in range(KC):
                        nc.tensor.matmul(
                            ph[:], w1t[:, k, m * 128:(m + 1) * 128],
                            zt[:, k, :], start=(k == 0), stop=(k == KC - 1))
                    nc.scalar.activation(h1[:, m, :], ph[:], AF.Relu,
                                         bias=b1t[:, m:m + 1])
                    yield
                st2 = sp.tile([128, 4 * G], F32, tag="st2", name="st2")
                y2w = []
                for w in range(G):
                    pf = pbig.tile([128, D], F32, tag="big", name="pf")
                    for m0 in range(0, FC, 4):
                        for m in range(m0, m0 + 4):
                            nc.tensor.matmul(
                                pf[:], h1[:, m, w * W:(w + 1) * W],
                                w2t[:, m, :], start=(m == 0), stop=False)
                        yield
                    nc.tensor.matmul(pf[:], ones1[:], b2r[:],
                                     start=False, stop=True)
                    y2g = sp.tile([128, D], F32, tag="y2g", name="y2g")
                    nc.vector.tensor_mul(y2g[:], zw[w][:], g1b[:])
                    y2 = sp.tile([128, D], F32, tag=f"y2_{w}",
                                 name=f"y2_{w}", bufs=1)
                    nc.vector.scalar_tensor_tensor(y2[:], pf[:], 0.0, y2g[:],
                                                   ALU.add, ALU.add,
                                                   accum_out=st2[:, w:w + 1])
                    sq2 = sp.tile([128, D], BF, tag="sq2", name="sq2")
                    nc.vector.scalar_tensor_tensor(sq2[:], y2[:], 0.0, y2[:],
                                                   ALU.add, ALU.mult,
                                                   accum_out=st2[:, G + w:G + w + 1])
                    y2w.append(y2)
                    yield
                batched_rstd(st2)
                yield
                for w in range(G):
                    z2 = sp.tile([128, D], F32, tag="z2", name="z2")
                    nc.scalar.activation(z2[:], y2w[w][:], AF.Identity,
                                         bias=st2[:, 3 * G + w:3 * G + w + 1],
                                         scale=st2[:, 2 * G + w:2 * G + w + 1])
                    yo = sp.tile([128, D], F32, tag="yo", name="yo")
                    nc.vector.tensor_mul(z2[:], z2[:], g2b[:])
                    nc.vector.tensor_add(yo[:], z2[:], bb2[:])
                    nc.sync.dma_start(ov[G * it + w], yo[:])
                    yield

            def drive(gen, n):
                if gen is None:
                    return
                for _ in range(n):
                    try:
                        next(gen)
                    except StopIteration:
                        return

            ffg = None
            for it in range(n_iters):
                # ---- loads + x transposes (f32 stationary, bf16 out) ----
                if it == 0:
                    x = x0
                else:
                    x = sp.tile([128, G, D], F32, tag="x")
                    for w in range(G):
                        nc.sync.dma_start(x[:, w, :], xv[G * it + w])
                xtp = sp.tile([128, KC, G * W], BF, tag="xtp")
                for w in range(G):
                    tpx = ptp.tile([128, KC, 128], F32, tag="tp")
                    for k in range(KC):
                        nc.tensor.transpose(tpx[:, k, :],
                                            x[:, w, k * 128:(k + 1) * 128],
                                            idf[:])
                    nc.vector.tensor_copy(xtp[:, :, w * W:(w + 1) * W], tpx[:])

                # ---- qkT (e-major) ----
                qkt = sp.tile([128, 8, G * W], BF, tag="qkt")
                for m in range(8):
                    pq = pbig.tile([128, G * W], F32, tag="big", name="pq")
                    for k in range(KC):
                        nc.tensor.matmul(
                            pq[:], wqk[:, k, m * 128:(m + 1) * 128],
                            xtp[:, k, :], start=(k == 0), stop=(k == KC - 1))
                    nc.scalar.activation(qkt[:, m, :], pq[:], AF.Identity,
                                         bias=qkb[:, m:m + 1])

                # ---- v token-major, with ones column per head ----
                v = sp.tile([128, G, H, VP], BF, tag="v")
                for w in range(G):
                    pv = pbig.tile([128, D], F32, tag="big", name="pv")
                    for k in range(KC):
                        nc.tensor.matmul(
                            pv[:], xtp[:, k, w * W:(w + 1) * W], wv[:, k, :],
                            start=(k == 0), stop=(k == KC - 1))
                    pvv = pv[:].rearrange("p (h e) -> p h e", h=H)
                    nc.vector.tensor_copy(v[:, w, :, 0:HD], pvv)
                    nc.vector.memset(v[:, w, :, HD:HD + 1], 1.0)

                # ---- scores (transposed), exp, attention + rowsum ----
                attok = [sp.tile([128, D], BF, tag=f"attok{w}", name=f"attok{w}")
                         for w in range(G)]
                for h in range(H):
                    j, pb = h // 2, (h % 2) * 64
                    sc = psc.tile([128, G * W], F32, tag="sc")
                    for w in range(G):
                        lk = qkt[pb:pb + 64, 4 + j, w * W:(w + 1) * W]
                        lq = qkt[pb:pb + 64, j, w * W:(w + 1) * W]
                        nc.tensor.matmul(sc[:, w * W:(w + 1) * W], lk, lq,
                                         start=True, stop=True,
                                         tile_position=(pb, 0))
                    pr = sp3.tile([128, G * W], BF, tag="pr")
                    nc.scalar.activation(pr[:], sc[:], AF.Exp)
                    drive(ffg, 1)
                    pat = ppat.tile([128, G, HD + 1], F32, tag="pat")
                    for w in range(G):
                        nc.tensor.matmul(
                            pat[:, w, :], pr[:, w * W:(w + 1) * W],
                            v[:, w, h, 0:HD + 1], start=True, stop=True)
                    rt = sp.tile([128, G, 1], F32, tag="rt")
                    nc.vector.reciprocal(rt[:], pat[:, :, HD:HD + 1])
                    for w in range(G):
                        nc.vector.tensor_scalar(
                            attok[w][:, h * HD:(h + 1) * HD],
                            pat[:, w, 0:HD], rt[:, w, :], None,
                            ALU.mult)
                    drive(ffg, 1)

                # ---- attn transpose + out-proj + residual ----
                zt = sp.tile([128, KC, G * W], BF, tag="zt")
                zw = [sp.tile([128, D], BF, tag=f"z{w}", name=f"z{w}")
                      for w in range(G)]
                st1 = sp.tile([128, 4 * G], F32, tag="st1")
                y1w = []
                for w in range(G):
                    tpa = ptp.tile([128, KC, 128], BF, tag="tp")
                    for k in range(KC):
                        nc.tensor.transpose(tpa[:, k, :],
                                            attok[w][:, k * 128:(k + 1) * 128],
                                            idb[:])
                    ats = sp.tile([128, KC, 128], BF, tag="ats")
                    nc.vector.tensor_copy(ats[:], tpa[:])
                    drive(ffg, 1)

                    pao = pbig.tile([128, D], F32, tag="big", name="pao")
                    for k in range(KC):
                        nc.tensor.matmul(pao[:], ats[:, k, :], wo[:, k, :],
                                         start=(k == 0), stop=(k == KC - 1))

                    xob = sp.tile([128, D], F32, tag="xob")
                    nc.vector.tensor_add(xob[:], x[:, w, :], obc[:])
                    y1 = sp.tile([128, D], F32, tag=f"y1_{w}",
                                 name=f"y1_{w}", bufs=1)
                    nc.vector.scalar_tensor_tensor(y1[:], pao[:], 0.0,
                                                   xob[:], ALU.add,
                                                   ALU.add,
                                                   accum_out=st1[:, w:w + 1])
                    sq = sp.tile([128, D], BF, tag="sq")
                    nc.vector.scalar_tensor_tensor(sq[:], y1[:], 0.0, y1[:],
                                                   ALU.add, ALU.mult,
                                                   accum_out=st1[:, G + w:G + w + 1])
                    y1w.append(y1)
                    drive(ffg, 2)

                # ---- LN1 + z transposes ----
                batched_rstd(st1)
                for w in range(G):
                    nc.scalar.activation(zw[w][:], y1w[w][:], AF.Identity,
                                         bias=st1[:, 3 * G + w:3 * G + w + 1],
                                         scale=st1[:, 2 * G + w:2 * G + w + 1])
                    tpz = ptp.tile([128, KC, 128], BF, tag="tp")
                    for k in range(KC):
                        nc.tensor.transpose(tpz[:, k, :],
                                            zw[w][:, k * 128:(k + 1) * 128],
                                            idb[:])
                    nc.vector.tensor_copy(zt[:, :, w * W:(w + 1) * W], tpz[:])
                    drive(ffg, 3)
                drive(ffg, 999)
                ffg = emit_ff(it, zt, zw)
            drive(ffg, 999)

    nc.compile()
    return nc


def _pack(wT, kc):
    """[kc*128, N] -> [128, kc*N] with partition p, block k = wT[k*128+p]."""
    n = wT.shape[1]
    return np.ascontiguousarray(
        wT.reshape(kc, 128, n).transpose(1, 0, 2).reshape(128, kc * n))


_CACHE = {}


def _get_nc(n_iters=ITERS):
    if n_iters not in _CACHE:
        _CACHE[n_iters] = _build_nc(n_iters)
    return _CACHE[n_iters]


def _prep_inputs(src, in_proj_w, in_proj_b, out_w, out_b, ln1_g, ln1_b,
                 w1, b1, w2, b2, ln2_g, ln2_b, n_iters=ITERS):
    src = np.asarray(src, np.float32)
    scale = 1.0 / np.sqrt(HD)

    in_proj_w = np.asarray(in_proj_w, np.float32)
    in_proj_b = np.asarray(in_proj_b, np.float32)
    out_w = np.asarray(out_w, np.float32)
    out_b = np.asarray(out_b, np.float32)
    w1 = np.asarray(w1, np.float32)
    b1 = np.asarray(b1, np.float32)
    w2 = np.asarray(w2, np.float32)
    b2 = np.asarray(b2, np.float32)
    ln1_g = np.asarray(ln1_g, np.float32)
    ln1_b = np.asarray(ln1_b, np.float32)
    ln2_g = np.asarray(ln2_g, np.float32)
    ln2_b = np.asarray(ln2_b, np.float32)

    # q scaled by 1/sqrt(HD); k-bias is softmax-invariant (constant along
    # the key axis for fixed q) and dropped.
    wqkT = in_proj_w[:2 * D].T.copy()       # [512, 1024], q cols then k cols
    wqkT[:, :D] *= scale
    bqk = np.zeros((2 * D,), np.float32)
    bqk[:D] = in_proj_b[:D] * scale

    bv = in_proj_b[2 * D:]
    obp = out_b + out_w @ bv                # fold v-bias through out proj
    w1f = w1 * ln1_g[None, :]               # fold LN1 gain into FF1
    b1f = b1 + w1 @ ln1_b                   # fold LN1 bias into FF1 bias
    b2f = b2 + ln1_b                        # fold LN1 bias into residual-2

    common = {
        "wqk": _pack(wqkT.astype(BF16), KC),
        "wv": _pack(in_proj_w[2 * D:].T.astype(BF16), KC),
        "wo": _pack(out_w.T.astype(BF16), KC),
        "w1t": _pack(w1f.T.astype(BF16), KC),
        "w2t": _pack(w2.T.astype(BF16), FC),
        "qkb": np.ascontiguousarray(bqk.reshape(8, 128).T),
        "b1t": np.ascontiguousarray(b1f.reshape(FC, 128).T),
        "obc": np.ascontiguousarray(np.broadcast_to(obp, (128, D))),
        "b2r": b2f.astype(BF16)[None, :],
        "g1b": np.ascontiguousarray(np.broadcast_to(ln1_g, (128, D))),
        "g2b": np.ascontiguousarray(np.broadcast_to(ln2_g, (128, D))),
        "bb2": np.ascontiguousarray(np.broadcast_to(ln2_b, (128, D))),
        "ones1": np.ones((1, 128), BF16),
        "idb": np.eye(128, dtype=BF16),
        "idf": np.eye(128, dtype=np.float32),
    }

    wins = src.reshape(NW_TOT, W, D)
    wpc = n_iters * G
    in_maps = []
    for c in range(N_CORES):
        m = dict(common)
        m["x"] = np.ascontiguousarray(
            wins[c * wpc:(c + 1) * wpc].reshape(wpc * W, D))
        in_maps.append(m)
    return in_maps


def kernel(src, in_proj_w, in_proj_b, out_w, out_b, ln1_g, ln1_b,
           w1, b1, w2, b2, ln2_g, ln2_b):
    nc = _get_nc()
    in_maps = _prep_inputs(src, in_proj_w, in_proj_b, out_w, out_b, ln1_g,
                           ln1_b, w1, b1, w2, b2, ln2_g, ln2_b)
    res = run_bass_kernel_spmd(nc, in_maps, list(range(N_CORES)))
    out = np.concatenate([res.results[c]["out"] for c in range(N_CORES)], axis=0)
    return np.ascontiguousarray(out.reshape(B, S, D)).astype(np.float32)
